# revision 24
# baseline (speedup 1.0000x reference)
"""Trainium2 Bass kernel for nn_CommBlock (gnn_message_passing).

Sharding: pure data-parallel over B=1024 across 8 cores (128 batch/core).

On-chip design (per core): all activations kept TRANSPOSED (feature dim on
partitions, node dim n on the free axis) so no on-chip transposes are needed.
Attention mask is applied by a VectorE multiply of exp(scores) with the
comm mask (shipped transposed as uint8, converted to bf16 on chip), which
yields exact zeros for blocked pairs.  Softmax denominators via a ones-vector
matmul (column-tiled 4x concurrent); division via reciprocal_approx_fast +
partition-broadcast DMA.  GRU biases are folded into a K=65-augmented Wih
matmul; sigmoid is computed as 0.5*tanh(0.5x)+0.5 so ScalarE needs only one
activation-table set (exp+tanh).  The update-mask blend is fused with the
(1-z) factor.  Output is returned in bf16 (halves the D2H and HBM-write
traffic); the final cast back to f32 happens on host.

build_bass(hw_loop=True, reps=R) wraps the whole per-core body in a For_i
hardware loop executing R identical sweeps — used by the benchmark harness
for repeat-differencing HW timing (NEFF size independent of R).
"""

import sys
import numpy as np

sys.path.insert(0, "/opt/trn_rl_repo")

import ml_dtypes

BF16 = ml_dtypes.bfloat16

B, N, D = 1024, 128, 256
H, DH = 4, 64
G3 = 3 * D  # 768
NCORES = 8
BC = B // NCORES  # batch per core (128)
G = 4  # batch-group size on chip


def build_bass(bc=BC, reps=1, hw_loop=False):
    import concourse.bass as bass
    import concourse.bass_isa as bass_isa
    import concourse.tile as tile
    from concourse import bacc, mybir

    f32 = mybir.dt.float32
    bf16 = mybir.dt.bfloat16
    u8 = mybir.dt.uint8
    AF = mybir.ActivationFunctionType
    ALU = mybir.AluOpType

    nc = bacc.Bacc()

    # ---- DRAM parameters (per-core shard; host pre-packs layouts) ----
    # latT/out_t/commT are partition-major so each per-group DMA reads one
    # contiguous slab per partition line.
    latT = nc.declare_dram_parameter("latT", [128, bc, 2, N], bf16, isOutput=False)
    commT = nc.declare_dram_parameter("commT", [128, bc, N], u8, isOutput=False)
    umask = nc.declare_dram_parameter("umask", [bc, N], bf16, isOutput=False)
    wq_t = nc.declare_dram_parameter("wq_t", [128, 2, 256], bf16, isOutput=False)
    wk_t = nc.declare_dram_parameter("wk_t", [128, 2, 256], bf16, isOutput=False)
    wv_t = nc.declare_dram_parameter("wv_t", [128, 2, 256], bf16, isOutput=False)
    wo_t = nc.declare_dram_parameter("wo_t", [128, 2, DH], bf16, isOutput=False)
    wih_aug = nc.declare_dram_parameter("wih_aug", [65, G3], bf16, isOutput=False)
    whh_t = nc.declare_dram_parameter("whh_t", [128, 2, G3], bf16, isOutput=False)
    bhh_n2 = nc.declare_dram_parameter("bhh_n2", [128, 2], f32, isOutput=False)
    out_t = nc.declare_dram_parameter("out_t", [128, bc, 2, N], bf16, isOutput=True)

    with tile.TileContext(nc) as tc:
        with (
            tc.tile_pool(name="consts", bufs=1) as consts,
            tc.tile_pool(name="state", bufs=2) as state,
            tc.tile_pool(name="work", bufs=2) as work,
            tc.tile_pool(name="gates", bufs=2) as gates,
            tc.tile_pool(name="outp", bufs=2) as outp,
            # Two PSUM pools, 8 banks total; tags are shared across phases so
            # sequential phases reuse the same banks.
            tc.tile_pool(name="dramp", bufs=2, space="DRAM") as dramp,
            tc.tile_pool(name="ps_big", bufs=1, space="PSUM") as ps_big,
            tc.tile_pool(name="ps_small", bufs=2, space="PSUM") as ps_small,
        ):
            # ---------------- constants ----------------
            wq = consts.tile([128, 2, 256], bf16)
            nc.sync.dma_start(out=wq, in_=wq_t[:])
            wk = consts.tile([128, 2, 256], bf16)
            nc.sync.dma_start(out=wk, in_=wk_t[:])
            wv = consts.tile([128, 2, 256], bf16)
            nc.sync.dma_start(out=wv, in_=wv_t[:])
            wo = consts.tile([128, 2, DH], bf16)
            nc.sync.dma_start(out=wo, in_=wo_t[:])
            wih = consts.tile([65, G3], bf16)
            nc.sync.dma_start(out=wih, in_=wih_aug[:])
            whh = consts.tile([128, 2, G3], bf16)
            nc.sync.dma_start(out=whh, in_=whh_t[:])
            bhh = consts.tile([128, 2], f32)
            nc.sync.dma_start(out=bhh, in_=bhh_n2[:])
            ones_col = consts.tile([128, 32], bf16)
            nc.vector.memset(ones_col, 1.0)

            def body():
                # ---------------- main loop over groups of G ----------------
                for g in range(bc // G):
                    lt = state.tile([128, G, 2, N], bf16, tag="lt")
                    um = state.tile([128, G, N], bf16, tag="um")
                    blk8 = state.tile([128, G, N], u8, tag="blk8")
                    blkb = state.tile([128, G, N], bf16, tag="blkb")
                    bg0 = g * G
                    # contiguous per-partition slabs: latT[d', bg:bg+G, :, :]
                    nc.sync.dma_start(
                        out=lt,
                        in_=bass.AP(tensor=latT, offset=bg0 * 2 * N,
                                    ap=[[bc * 2 * N, 128], [2 * N, G], [N, 2],
                                        [1, N]]))
                    nc.sync.dma_start(
                        out=um,
                        in_=bass.AP(tensor=umask, offset=umask[bg0].offset,
                                    ap=[[0, 128], [N, G], [1, N]]))
                    # blk8[m, b, n] <- commT[m, bg+b, n]  (keep mask, 1=allowed)
                    nc.sync.dma_start(
                        out=blk8,
                        in_=bass.AP(tensor=commT, offset=bg0 * N,
                                    ap=[[bc * N, 128], [N, G], [1, N]]))
                    nc.scalar.copy(blkb, blk8)

                    outt = outp.tile([128, G, 2, N], bf16, tag="outt")

                    for layer in range(2):
                        # ---------- projections (group-wide) ----------
                        qt_ps = ps_big.tile([128, 2, G * N], f32, tag="pbA")
                        kt_ps = ps_big.tile([128, 2, G * N], f32, tag="pbB")
                        v_ps = ps_big.tile([128, G, 256], f32, tag="pbC")
                        for jblk in range(2):
                            for kblk in range(2):
                                nc.tensor.matmul(
                                    qt_ps[:, jblk, :],
                                    wq[:, kblk, jblk * 128:(jblk + 1) * 128],
                                    lt.rearrange("d b k n -> d k b n")[:, kblk, :, :],
                                    start=(kblk == 0), stop=(kblk == 1))
                                nc.tensor.matmul(
                                    kt_ps[:, jblk, :],
                                    wk[:, kblk, jblk * 128:(jblk + 1) * 128],
                                    lt.rearrange("d b k n -> d k b n")[:, kblk, :, :],
                                    start=(kblk == 0), stop=(kblk == 1))
                        for b in range(G):
                            for kblk in range(2):
                                nc.tensor.matmul(
                                    v_ps[:, b, :],
                                    lt[:, b, kblk, :],
                                    wv[:, kblk, :],
                                    start=(kblk == 0), stop=(kblk == 1))
                        qt = work.tile([128, 2, G * N], bf16, tag="qt")
                        kt = work.tile([128, 2, G * N], bf16, tag="kt")
                        v = work.tile([128, G, 256], bf16, tag="v")
                        # upper halves first: they feed the head-remap DMA
                        nc.vector.tensor_copy(qt[64:128, :, :], qt_ps[64:128, :, :])
                        nc.scalar.copy(kt[64:128, :, :], kt_ps[64:128, :, :])
                        nc.vector.tensor_copy(qt[0:64, :, :], qt_ps[0:64, :, :])
                        nc.scalar.copy(kt[0:64, :, :], kt_ps[0:64, :, :])
                        nc.scalar.copy(v, v_ps)
                        # heads {0,2} live at partitions 0:64 of qt/kt jblk 0/1
                        # and are read directly (PE cannot read operands at
                        # partition base 64 -> crashes device).  Only heads
                        # {1,3} (partitions 64:128) are remapped down, one
                        # DMA per tensor: hi[d,t,hh,:] <- {qt,kt}[64+d,hh,:].
                        hi = work.tile([64, 2, 2, G * N], bf16, tag="hi")
                        nc.sync.dma_start(out=hi[:, 0, :, :],
                                          in_=qt[64:128, :, :])
                        nc.sync.dma_start(out=hi[:, 1, :, :],
                                          in_=kt[64:128, :, :])

                        # ---------- attention ----------
                        e = work.tile([128, G, H * N], bf16, tag="e")
                        den_ps = ps_big.tile([128, 4 * N], f32, tag="pbC")
                        for b in range(G):
                            bs0, bs1 = b * N, (b + 1) * N
                            sc_ps = ps_small.tile([128, H, N], f32, tag="psA")
                            # heads 0,2 direct; heads 1,3 from the remap tile
                            plan = [
                                (0, kt[0:64, 0, bs0:bs1], qt[0:64, 0, bs0:bs1]),
                                (2, kt[0:64, 1, bs0:bs1], qt[0:64, 1, bs0:bs1]),
                                (1, hi[:, 1, 0, bs0:bs1], hi[:, 0, 0, bs0:bs1]),
                                (3, hi[:, 1, 1, bs0:bs1], hi[:, 0, 1, bs0:bs1]),
                            ]
                            for idx, (h, kap, qap) in enumerate(plan):
                                nc.tensor.matmul(
                                    sc_ps[:, h, :], kap, qap,
                                    start=(idx == 0), stop=(idx == 3))
                            nc.scalar.activation(
                                e[:, b, :], sc_ps.rearrange("m h n -> m (h n)"),
                                AF.Exp)
                            # mask: e *= comm[m, n] (broadcast over heads)
                            eb = bass.AP(
                                tensor=e.tensor, offset=e[:, b, :].offset,
                                ap=[list(e.ap[0]), [N, H], [1, N]])
                            mb_ap = bass.AP(
                                tensor=blkb.tensor,
                                offset=blkb[:, b, :].offset,
                                ap=[list(blkb.ap[0]), [0, H], [1, N]])
                            nc.vector.tensor_mul(eb, eb, mb_ap)
                            # denominators -> 32 copies at partition 32*b
                            nc.tensor.matmul(
                                den_ps[32 * b:32 * b + 32, :],
                                ones_col,
                                e[:, b, :],
                                start=True, stop=True,
                                tile_position=(0, 32 * b))
                        recip_f = work.tile([128, 4 * N], f32, tag="recip_f")
                        nc.vector.reciprocal_approx_fast(
                            out=recip_f[0:97, :], in_=den_ps[0:97, :])
                        recip = work.tile([128, 4 * N], bf16, tag="recip")
                        nc.vector.tensor_copy(recip[0:97, :], recip_f[0:97, :])
                        rscr = dramp.tile([G, H * N], bf16, tag="rscr")
                        nc.sync.dma_start(out=rscr, in_=recip[::32, :])
                        rb = work.tile([128, G, H * N], bf16, tag="rb")
                        for b in range(G):
                            nc.sync.dma_start(
                                out=rb[:, b, :],
                                in_=bass.AP(tensor=rscr.tensor, offset=rscr[b].offset,
                                            ap=[[0, 128], [1, H * N]]))

                        # ---------- ctx (heads column-packed in pairs) ----------
                        # matmuls consume unnormalized e; the 1/den factor is
                        # folded into the PSUM->SBUF copy (free on DVE).
                        ctxs = work.tile([128, 2, G, N], bf16, tag="ctxs")
                        for b in range(G):
                            ctx_ps = ps_small.tile([128, 4, N], f32, tag="psA")
                            for h in range(H):
                                jb, off = h // 2, (h % 2) * 64
                                nc.tensor.matmul(
                                    ctx_ps[off:off + 64, jb, :],
                                    v[:, b, h * 64:(h + 1) * 64],
                                    e[:, b, h * N:(h + 1) * N],
                                    start=(h < 2), stop=(h >= 2),
                                    skip_group_check=True)
                            # normalized copy: head = 2*jb + (p >= 64)
                            for half in range(2):
                                sl = slice(64 * half, 64 * half + 64)
                                rv = bass.AP(
                                    tensor=rb.tensor,
                                    offset=rb[sl, b, half * N].offset,
                                    ap=[[rb.ap[0][0], 64], [2 * N, 2], [1, N]])
                                nc.vector.tensor_mul(
                                    ctxs[sl, :, b, :], ctx_ps[sl, 0:2, :], rv)

                        # ---------- info^T (M=64) + ones augmentation ----------
                        info_ps = ps_big.tile([64, G, N], f32, tag="pbC")
                        for b in range(G):
                            for jblk in range(2):
                                nc.tensor.matmul(
                                    info_ps[:, b, :],
                                    wo[:, jblk, :],
                                    ctxs[:, jblk, b, :],
                                    start=(jblk == 0), stop=(jblk == 1))
                        infoa = work.tile([65, G, N], bf16, tag="infoa")
                        nc.vector.memset(infoa[64:65, :, :], 1.0)
                        nc.scalar.copy(infoa[0:64, :, :], info_ps)

                        # ---------- GRU gates, per batch element ----------
                        # per-elem 1-bank PSUM tiles from the ps_small ring so
                        # pbA/pbB (q/k) stay free for the next layer's
                        # projections to overlap with the GRU tail.
                        for b in range(G):
                            grz_ps = ps_small.tile([128, 4, N], f32, tag="psA")
                            gn_ps = ps_small.tile([128, 4, N], f32, tag="psA")
                            for mb in range(4):
                                for kblk in range(2):
                                    nc.tensor.matmul(
                                        grz_ps[:, mb, :],
                                        whh[:, kblk, mb * 128:(mb + 1) * 128],
                                        lt[:, b, kblk, :],
                                        start=(kblk == 0), stop=False)
                                nc.tensor.matmul(
                                    grz_ps[:, mb, :],
                                    wih[:, mb * 128:(mb + 1) * 128],
                                    infoa[:, b, :],
                                    start=False, stop=True)
                            for i in range(2):
                                mb = 4 + i
                                nc.tensor.matmul(
                                    gn_ps[:, i, :],
                                    wih[:, mb * 128:(mb + 1) * 128],
                                    infoa[:, b, :],
                                    start=True, stop=True)
                                for kblk in range(2):
                                    nc.tensor.matmul(
                                        gn_ps[:, 2 + i, :],
                                        whh[:, kblk, mb * 128:(mb + 1) * 128],
                                        lt[:, b, kblk, :],
                                        start=(kblk == 0), stop=(kblk == 1))
                            # t = tanh(0.5*g_rz)  (biases already in psum)
                            trz = gates.tile([128, 4, N], bf16, tag="trz")
                            nc.scalar.activation(trz, grz_ps, AF.Tanh, scale=0.5)
                            # r = 0.5*t_r + 0.5
                            r = gates.tile([128, 2, N], bf16, tag="r")
                            nc.vector.tensor_scalar(
                                out=r, in0=trz[:, 0:2, :], scalar1=0.5, scalar2=0.5,
                                op0=ALU.mult, op1=ALU.add)
                            # rhn = (gh_n + bhh_n) * r
                            rhn = gates.tile([128, 2, N], bf16, tag="rhn")
                            for i in range(2):
                                nc.vector.scalar_tensor_tensor(
                                    out=rhn[:, i, :], in0=gn_ps[:, 2 + i, :],
                                    scalar=bhh[:, i:i + 1], in1=r[:, i, :],
                                    op0=ALU.add, op1=ALU.mult)
                            # nn = tanh(gi_n + rhn)
                            nna = gates.tile([128, 2, N], bf16, tag="nna")
                            nc.vector.tensor_add(nna, gn_ps[:, 0:2, :], rhn)
                            nn = gates.tile([128, 2, N], bf16, tag="nn")
                            nc.scalar.activation(nn, nna, AF.Tanh)
                            # zc = umask*(1-z);  1-z = 0.5 - 0.5*t_z
                            zcn = gates.tile([128, 2, N], bf16, tag="zcn")
                            nc.vector.tensor_scalar(
                                out=zcn, in0=trz[:, 2:4, :], scalar1=-0.5,
                                scalar2=0.5, op0=ALU.mult, op1=ALU.add)
                            zc = gates.tile([128, 2, N], bf16, tag="zc")
                            umb = um[:, b, :]
                            nc.vector.tensor_mul(
                                zc,
                                zcn,
                                bass.AP(tensor=umb.tensor, offset=umb.offset,
                                        ap=[umb.ap[0], [0, 2], [1, N]]))
                            # h' = lt + zc*(nn - lt)
                            lts = lt[:, b, :, :]
                            w3 = gates.tile([128, 2, N], bf16, tag="w3")
                            nc.vector.tensor_sub(w3, nn, lts)
                            v3 = gates.tile([128, 2, N], bf16, tag="v3")
                            nc.vector.tensor_mul(v3, w3, zc)
                            if layer == 0:
                                nc.vector.tensor_add(lts, lts, v3)
                            else:
                                nc.vector.tensor_add(outt[:, b, :, :], lts, v3)

                    nc.sync.dma_start(
                        out=bass.AP(tensor=out_t, offset=bg0 * 2 * N,
                                    ap=[[bc * 2 * N, 128], [2 * N, G], [N, 2],
                                        [1, N]]),
                        in_=outt)

            if hw_loop:
                with tc.For_i(0, reps):
                    body()
            else:
                for _ in range(reps):
                    body()

    nc.compile()
    return nc


def prep_inputs(inputs, bc=BC, ncores=NCORES):
    latent = np.asarray(inputs["latent"], np.float32)
    comm = np.asarray(inputs["comm_mask"])
    Wq = np.asarray(inputs["Wq"], np.float32)
    Wk = np.asarray(inputs["Wk"], np.float32)
    Wv = np.asarray(inputs["Wv"], np.float32)
    Wo = np.asarray(inputs["Wo"], np.float32)
    Wih = np.asarray(inputs["Wih"], np.float32)
    Whh = np.asarray(inputs["Whh"], np.float32)
    bih = np.asarray(inputs["bih"], np.float32)
    bhh = np.asarray(inputs["bhh"], np.float32)

    scale = 1.0 / np.sqrt(DH)
    nb = bc * ncores
    # [b, n, d] -> [d', b, k, n] with d = k*128 + d'  (partition-major)
    latT = np.ascontiguousarray(
        latent[:nb].reshape(nb, N, 2, 128).transpose(3, 0, 2, 1)
    ).astype(BF16)
    # keep mask, partition-major transposed: commT[m, b, n] = comm[b, n, m]
    commT = np.ascontiguousarray(comm[:nb].transpose(2, 0, 1)).astype(np.uint8)
    umask = (comm[:nb].sum(-1) > 1).astype(np.float32).astype(BF16)  # [b, n]

    def wt(w, s=1.0):  # [j, d] -> [d', k, j]
        j = w.shape[0]
        return np.ascontiguousarray(
            (w.T * s).reshape(2, 128, j).transpose(1, 0, 2)).astype(BF16)

    bias_g = bih + bhh
    bias_g[2 * D:] = bih[2 * D:]
    wih_aug = np.concatenate([Wih.T, bias_g[None, :]], 0).astype(BF16)  # [65, 768]
    bhh_n2 = np.ascontiguousarray(bhh[2 * D:].reshape(2, 128).T).astype(np.float32)

    shared = {
        "wq_t": wt(Wq, scale), "wk_t": wt(Wk), "wv_t": wt(Wv), "wo_t": wt(Wo),
        "wih_aug": wih_aug, "whh_t": wt(Whh), "bhh_n2": bhh_n2,
    }
    in_maps = []
    for c in range(ncores):
        sl = slice(c * bc, (c + 1) * bc)
        in_maps.append({
            "latT": np.ascontiguousarray(latT[:, sl]),
            "commT": np.ascontiguousarray(commT[:, sl]),
            "umask": umask[sl],
            **shared,
        })
    return in_maps


def unpack_out(o, bc=BC):
    # [128, bc, 2, N] bf16 -> [bc, N, D] f32
    return o.astype(np.float32).transpose(1, 3, 2, 0).reshape(bc, N, D)


_NC_CACHE = None


def kernel(**inputs) -> np.ndarray:
    global _NC_CACHE
    from concourse.bass_utils import run_bass_kernel_spmd

    bq = np.asarray(inputs["bq"]); bk = np.asarray(inputs["bk"])
    bv = np.asarray(inputs["bv"])
    assert not np.any(bq) and not np.any(bk) and not np.any(bv), \
        "kernel assumes zero qkv biases"

    if _NC_CACHE is None:
        _NC_CACHE = build_bass()
    in_maps = prep_inputs(inputs)
    res = run_bass_kernel_spmd(_NC_CACHE, in_maps, list(range(NCORES)))
    outs = [unpack_out(res.results[c]["out_t"]) for c in range(NCORES)]
    return np.ascontiguousarray(np.concatenate(outs, 0)).astype(np.float32)


# revision 25
# speedup vs baseline: 1.0531x; 1.0531x over previous
"""Trainium2 Bass kernel for nn_CommBlock (gnn_message_passing).

Sharding: pure data-parallel over B=1024 across 8 cores (128 batch/core).

On-chip design (per core): all activations kept TRANSPOSED (feature dim on
partitions, node dim n on the free axis) so no on-chip transposes are needed.
Attention mask is applied by a VectorE multiply of exp(scores) with the
comm mask (shipped transposed as uint8, converted to bf16 on chip), which
yields exact zeros for blocked pairs.  Softmax denominators via a ones-vector
matmul (column-tiled 4x concurrent); division via reciprocal_approx_fast +
partition-broadcast DMA.  GRU biases are folded into a K=65-augmented Wih
matmul; sigmoid is computed as 0.5*tanh(0.5x)+0.5 so ScalarE needs only one
activation-table set (exp+tanh).  The update-mask blend is fused with the
(1-z) factor.  Output is returned in bf16 (halves the D2H and HBM-write
traffic); the final cast back to f32 happens on host.

build_bass(hw_loop=True, reps=R) wraps the whole per-core body in a For_i
hardware loop executing R identical sweeps — used by the benchmark harness
for repeat-differencing HW timing (NEFF size independent of R).
"""

import sys
import numpy as np

sys.path.insert(0, "/opt/trn_rl_repo")

import ml_dtypes

BF16 = ml_dtypes.bfloat16

B, N, D = 1024, 128, 256
H, DH = 4, 64
G3 = 3 * D  # 768
NCORES = 8
BC = B // NCORES  # batch per core (128)
G = 4  # batch-group size on chip


def build_bass(bc=BC, reps=1, hw_loop=False):
    import concourse.bass as bass
    import concourse.bass_isa as bass_isa
    import concourse.tile as tile
    from concourse import bacc, mybir

    f32 = mybir.dt.float32
    bf16 = mybir.dt.bfloat16
    u8 = mybir.dt.uint8
    AF = mybir.ActivationFunctionType
    ALU = mybir.AluOpType

    nc = bacc.Bacc()

    # ---- DRAM parameters (per-core shard; host pre-packs layouts) ----
    # latT/out_t/commT are partition-major so each per-group DMA reads one
    # contiguous slab per partition line.
    latT = nc.declare_dram_parameter("latT", [128, bc, 2, N], bf16, isOutput=False)
    commT = nc.declare_dram_parameter("commT", [128, bc, N], u8, isOutput=False)
    umask = nc.declare_dram_parameter("umask", [bc, N], bf16, isOutput=False)
    wq_t = nc.declare_dram_parameter("wq_t", [128, 2, 256], bf16, isOutput=False)
    wk_t = nc.declare_dram_parameter("wk_t", [128, 2, 256], bf16, isOutput=False)
    wv_t = nc.declare_dram_parameter("wv_t", [128, 2, 256], bf16, isOutput=False)
    wo_t = nc.declare_dram_parameter("wo_t", [128, 2, DH], bf16, isOutput=False)
    wih_aug = nc.declare_dram_parameter("wih_aug", [65, G3], bf16, isOutput=False)
    whh_t = nc.declare_dram_parameter("whh_t", [128, 2, G3], bf16, isOutput=False)
    bhh_n2 = nc.declare_dram_parameter("bhh_n2", [128, 2], f32, isOutput=False)
    out_t = nc.declare_dram_parameter("out_t", [128, bc, 2, N], bf16, isOutput=True)

    with tile.TileContext(nc) as tc:
        with (
            tc.tile_pool(name="consts", bufs=1) as consts,
            tc.tile_pool(name="state", bufs=2) as state,
            tc.tile_pool(name="work", bufs=2) as work,
            tc.tile_pool(name="gates", bufs=2) as gates,
            tc.tile_pool(name="outp", bufs=2) as outp,
            # Two PSUM pools, 8 banks total; tags are shared across phases so
            # sequential phases reuse the same banks.
            tc.tile_pool(name="dramp", bufs=2, space="DRAM") as dramp,
            tc.tile_pool(name="ps_big", bufs=1, space="PSUM") as ps_big,
            tc.tile_pool(name="ps_small", bufs=2, space="PSUM") as ps_small,
        ):
            # ---------------- constants ----------------
            wq = consts.tile([128, 2, 256], bf16)
            nc.sync.dma_start(out=wq, in_=wq_t[:])
            wk = consts.tile([128, 2, 256], bf16)
            nc.sync.dma_start(out=wk, in_=wk_t[:])
            wv = consts.tile([128, 2, 256], bf16)
            nc.sync.dma_start(out=wv, in_=wv_t[:])
            wo = consts.tile([128, 2, DH], bf16)
            nc.sync.dma_start(out=wo, in_=wo_t[:])
            wih = consts.tile([65, G3], bf16)
            nc.sync.dma_start(out=wih, in_=wih_aug[:])
            whh = consts.tile([128, 2, G3], bf16)
            nc.sync.dma_start(out=whh, in_=whh_t[:])
            bhh = consts.tile([128, 2], f32)
            nc.sync.dma_start(out=bhh, in_=bhh_n2[:])
            ones_col = consts.tile([128, 32], bf16)
            nc.vector.memset(ones_col, 1.0)

            def body():
                # ---------------- main loop over groups of G ----------------
                for g in range(bc // G):
                    lt = state.tile([128, G, 2, N], bf16, tag="lt")
                    um = state.tile([128, G, N], bf16, tag="um")
                    blk8 = state.tile([128, G, N], u8, tag="blk8")
                    blkb = state.tile([128, G, N], bf16, tag="blkb")
                    bg0 = g * G
                    # contiguous per-partition slabs: latT[d', bg:bg+G, :, :]
                    nc.sync.dma_start(
                        out=lt,
                        in_=bass.AP(tensor=latT, offset=bg0 * 2 * N,
                                    ap=[[bc * 2 * N, 128], [2 * N, G], [N, 2],
                                        [1, N]]))
                    nc.sync.dma_start(
                        out=um,
                        in_=bass.AP(tensor=umask, offset=umask[bg0].offset,
                                    ap=[[0, 128], [N, G], [1, N]]))
                    # blk8[m, b, n] <- commT[m, bg+b, n]  (keep mask, 1=allowed)
                    nc.sync.dma_start(
                        out=blk8,
                        in_=bass.AP(tensor=commT, offset=bg0 * N,
                                    ap=[[bc * N, 128], [N, G], [1, N]]))
                    nc.scalar.copy(blkb, blk8)

                    outt = outp.tile([128, G, 2, N], bf16, tag="outt")

                    for layer in range(2):
                        # ---------- projections (group-wide) ----------
                        qt_ps = ps_big.tile([128, 2, G * N], f32, tag="pbA")
                        kt_ps = ps_big.tile([128, 2, G * N], f32, tag="pbB")
                        v_ps = ps_big.tile([128, G, 256], f32, tag="pbC")
                        for jblk in range(2):
                            for kblk in range(2):
                                nc.tensor.matmul(
                                    qt_ps[:, jblk, :],
                                    wq[:, kblk, jblk * 128:(jblk + 1) * 128],
                                    lt.rearrange("d b k n -> d k b n")[:, kblk, :, :],
                                    start=(kblk == 0), stop=(kblk == 1))
                                nc.tensor.matmul(
                                    kt_ps[:, jblk, :],
                                    wk[:, kblk, jblk * 128:(jblk + 1) * 128],
                                    lt.rearrange("d b k n -> d k b n")[:, kblk, :, :],
                                    start=(kblk == 0), stop=(kblk == 1))
                        for b in range(G):
                            for kblk in range(2):
                                nc.tensor.matmul(
                                    v_ps[:, b, :],
                                    lt[:, b, kblk, :],
                                    wv[:, kblk, :],
                                    start=(kblk == 0), stop=(kblk == 1))
                        qt = work.tile([128, 2, G * N], bf16, tag="qt")
                        kt = work.tile([128, 2, G * N], bf16, tag="kt")
                        v = work.tile([128, G, 256], bf16, tag="v")
                        nc.vector.tensor_copy(qt, qt_ps)
                        nc.scalar.copy(kt, kt_ps)
                        nc.scalar.copy(v, v_ps)
                        # heads {0,2} live at partitions 0:64 of qt/kt jblk 0/1
                        # and are read directly (PE cannot read operands at
                        # partition base 64 -> crashes device).  Only heads
                        # {1,3} (partitions 64:128) are remapped down, one
                        # DMA per tensor: hi[d,t,hh,:] <- {qt,kt}[64+d,hh,:].
                        hi = work.tile([64, 2, 2, G * N], bf16, tag="hi")
                        nc.sync.dma_start(out=hi[:, 0, :, :],
                                          in_=qt[64:128, :, :])
                        nc.sync.dma_start(out=hi[:, 1, :, :],
                                          in_=kt[64:128, :, :])

                        # ---------- attention ----------
                        e = work.tile([128, G, H * N], bf16, tag="e")
                        den_ps = ps_big.tile([128, 4 * N], f32, tag="pbC")
                        for b in range(G):
                            bs0, bs1 = b * N, (b + 1) * N
                            sc_ps = ps_small.tile([128, H, N], f32, tag="psA")
                            # heads 0,2 direct; heads 1,3 from the remap tile
                            plan = [
                                (0, kt[0:64, 0, bs0:bs1], qt[0:64, 0, bs0:bs1]),
                                (2, kt[0:64, 1, bs0:bs1], qt[0:64, 1, bs0:bs1]),
                                (1, hi[:, 1, 0, bs0:bs1], hi[:, 0, 0, bs0:bs1]),
                                (3, hi[:, 1, 1, bs0:bs1], hi[:, 0, 1, bs0:bs1]),
                            ]
                            for idx, (h, kap, qap) in enumerate(plan):
                                nc.tensor.matmul(
                                    sc_ps[:, h, :], kap, qap,
                                    start=(idx == 0), stop=(idx == 3))
                            nc.scalar.activation(
                                e[:, b, :], sc_ps.rearrange("m h n -> m (h n)"),
                                AF.Exp)
                            # mask: e *= comm[m, n] (broadcast over heads)
                            eb = bass.AP(
                                tensor=e.tensor, offset=e[:, b, :].offset,
                                ap=[list(e.ap[0]), [N, H], [1, N]])
                            mb_ap = bass.AP(
                                tensor=blkb.tensor,
                                offset=blkb[:, b, :].offset,
                                ap=[list(blkb.ap[0]), [0, H], [1, N]])
                            nc.vector.tensor_mul(eb, eb, mb_ap)
                            # denominators -> 32 copies at partition 32*b
                            nc.tensor.matmul(
                                den_ps[32 * b:32 * b + 32, :],
                                ones_col,
                                e[:, b, :],
                                start=True, stop=True,
                                tile_position=(0, 32 * b))
                        recip_f = work.tile([128, 4 * N], f32, tag="recip_f")
                        nc.vector.reciprocal_approx_fast(
                            out=recip_f[0:97, :], in_=den_ps[0:97, :])
                        recip = work.tile([128, 4 * N], bf16, tag="recip")
                        nc.vector.tensor_copy(recip[0:97, :], recip_f[0:97, :])
                        rscr = dramp.tile([G, H * N], bf16, tag="rscr")
                        nc.sync.dma_start(out=rscr, in_=recip[::32, :])
                        rb = work.tile([128, G, H * N], bf16, tag="rb")
                        for b in range(G):
                            nc.sync.dma_start(
                                out=rb[:, b, :],
                                in_=bass.AP(tensor=rscr.tensor, offset=rscr[b].offset,
                                            ap=[[0, 128], [1, H * N]]))

                        # ---------- ctx (heads column-packed in pairs) ----------
                        # matmuls consume unnormalized e; the 1/den factor is
                        # folded into the PSUM->SBUF copy (free on DVE).
                        ctxs = work.tile([128, 2, G, N], bf16, tag="ctxs")
                        for b in range(G):
                            ctx_ps = ps_small.tile([128, 4, N], f32, tag="psA")
                            for h in range(H):
                                jb, off = h // 2, (h % 2) * 64
                                nc.tensor.matmul(
                                    ctx_ps[off:off + 64, jb, :],
                                    v[:, b, h * 64:(h + 1) * 64],
                                    e[:, b, h * N:(h + 1) * N],
                                    start=(h < 2), stop=(h >= 2),
                                    skip_group_check=True)
                            # normalized copy: head = 2*jb + (p >= 64)
                            for half in range(2):
                                sl = slice(64 * half, 64 * half + 64)
                                rv = bass.AP(
                                    tensor=rb.tensor,
                                    offset=rb[sl, b, half * N].offset,
                                    ap=[[rb.ap[0][0], 64], [2 * N, 2], [1, N]])
                                nc.vector.tensor_mul(
                                    ctxs[sl, :, b, :], ctx_ps[sl, 0:2, :], rv)

                        # ---------- info^T (M=64) + ones augmentation ----------
                        info_ps = ps_big.tile([64, G, N], f32, tag="pbC")
                        for b in range(G):
                            for jblk in range(2):
                                nc.tensor.matmul(
                                    info_ps[:, b, :],
                                    wo[:, jblk, :],
                                    ctxs[:, jblk, b, :],
                                    start=(jblk == 0), stop=(jblk == 1))
                        infoa = work.tile([65, G, N], bf16, tag="infoa")
                        nc.vector.memset(infoa[64:65, :, :], 1.0)
                        nc.scalar.copy(infoa[0:64, :, :], info_ps)

                        # ---------- GRU gates, per batch element ----------
                        # per-elem 1-bank PSUM tiles from the ps_small ring so
                        # pbA/pbB (q/k) stay free for the next layer's
                        # projections to overlap with the GRU tail.
                        for b in range(G):
                            grz_ps = ps_small.tile([128, 4, N], f32, tag="psA")
                            gn_ps = ps_small.tile([128, 4, N], f32, tag="psA")
                            for mb in range(4):
                                for kblk in range(2):
                                    nc.tensor.matmul(
                                        grz_ps[:, mb, :],
                                        whh[:, kblk, mb * 128:(mb + 1) * 128],
                                        lt[:, b, kblk, :],
                                        start=(kblk == 0), stop=False)
                                nc.tensor.matmul(
                                    grz_ps[:, mb, :],
                                    wih[:, mb * 128:(mb + 1) * 128],
                                    infoa[:, b, :],
                                    start=False, stop=True)
                            for i in range(2):
                                mb = 4 + i
                                nc.tensor.matmul(
                                    gn_ps[:, i, :],
                                    wih[:, mb * 128:(mb + 1) * 128],
                                    infoa[:, b, :],
                                    start=True, stop=True)
                                for kblk in range(2):
                                    nc.tensor.matmul(
                                        gn_ps[:, 2 + i, :],
                                        whh[:, kblk, mb * 128:(mb + 1) * 128],
                                        lt[:, b, kblk, :],
                                        start=(kblk == 0), stop=(kblk == 1))
                            # t = tanh(0.5*g_rz)  (biases already in psum)
                            trz = gates.tile([128, 4, N], bf16, tag="trz")
                            nc.scalar.activation(trz, grz_ps, AF.Tanh, scale=0.5)
                            # r = 0.5*t_r + 0.5
                            r = gates.tile([128, 2, N], bf16, tag="r")
                            nc.vector.tensor_scalar(
                                out=r, in0=trz[:, 0:2, :], scalar1=0.5, scalar2=0.5,
                                op0=ALU.mult, op1=ALU.add)
                            # rhn = (gh_n + bhh_n) * r
                            rhn = gates.tile([128, 2, N], bf16, tag="rhn")
                            for i in range(2):
                                nc.vector.scalar_tensor_tensor(
                                    out=rhn[:, i, :], in0=gn_ps[:, 2 + i, :],
                                    scalar=bhh[:, i:i + 1], in1=r[:, i, :],
                                    op0=ALU.add, op1=ALU.mult)
                            # nn = tanh(gi_n + rhn)
                            nna = gates.tile([128, 2, N], bf16, tag="nna")
                            nc.vector.tensor_add(nna, gn_ps[:, 0:2, :], rhn)
                            nn = gates.tile([128, 2, N], bf16, tag="nn")
                            nc.scalar.activation(nn, nna, AF.Tanh)
                            # zc = umask*(1-z);  1-z = 0.5 - 0.5*t_z
                            zcn = gates.tile([128, 2, N], bf16, tag="zcn")
                            nc.vector.tensor_scalar(
                                out=zcn, in0=trz[:, 2:4, :], scalar1=-0.5,
                                scalar2=0.5, op0=ALU.mult, op1=ALU.add)
                            zc = gates.tile([128, 2, N], bf16, tag="zc")
                            umb = um[:, b, :]
                            nc.vector.tensor_mul(
                                zc,
                                zcn,
                                bass.AP(tensor=umb.tensor, offset=umb.offset,
                                        ap=[umb.ap[0], [0, 2], [1, N]]))
                            # h' = lt + zc*(nn - lt)
                            lts = lt[:, b, :, :]
                            w3 = gates.tile([128, 2, N], bf16, tag="w3")
                            nc.vector.tensor_sub(w3, nn, lts)
                            v3 = gates.tile([128, 2, N], bf16, tag="v3")
                            nc.vector.tensor_mul(v3, w3, zc)
                            if layer == 0:
                                nc.vector.tensor_add(lts, lts, v3)
                            else:
                                nc.vector.tensor_add(outt[:, b, :, :], lts, v3)

                    nc.sync.dma_start(
                        out=bass.AP(tensor=out_t, offset=bg0 * 2 * N,
                                    ap=[[bc * 2 * N, 128], [2 * N, G], [N, 2],
                                        [1, N]]),
                        in_=outt)

            if hw_loop:
                with tc.For_i(0, reps):
                    body()
            else:
                for _ in range(reps):
                    body()

    nc.compile()
    return nc


def prep_inputs(inputs, bc=BC, ncores=NCORES):
    latent = np.asarray(inputs["latent"], np.float32)
    comm = np.asarray(inputs["comm_mask"])
    Wq = np.asarray(inputs["Wq"], np.float32)
    Wk = np.asarray(inputs["Wk"], np.float32)
    Wv = np.asarray(inputs["Wv"], np.float32)
    Wo = np.asarray(inputs["Wo"], np.float32)
    Wih = np.asarray(inputs["Wih"], np.float32)
    Whh = np.asarray(inputs["Whh"], np.float32)
    bih = np.asarray(inputs["bih"], np.float32)
    bhh = np.asarray(inputs["bhh"], np.float32)

    scale = 1.0 / np.sqrt(DH)
    nb = bc * ncores
    # [b, n, d] -> [d', b, k, n] with d = k*128 + d'  (partition-major)
    latT = np.ascontiguousarray(
        latent[:nb].reshape(nb, N, 2, 128).transpose(3, 0, 2, 1)
    ).astype(BF16)
    # keep mask, partition-major transposed: commT[m, b, n] = comm[b, n, m]
    commT = np.ascontiguousarray(comm[:nb].transpose(2, 0, 1)).astype(np.uint8)
    umask = (comm[:nb].sum(-1) > 1).astype(np.float32).astype(BF16)  # [b, n]

    def wt(w, s=1.0):  # [j, d] -> [d', k, j]
        j = w.shape[0]
        return np.ascontiguousarray(
            (w.T * s).reshape(2, 128, j).transpose(1, 0, 2)).astype(BF16)

    bias_g = bih + bhh
    bias_g[2 * D:] = bih[2 * D:]
    wih_aug = np.concatenate([Wih.T, bias_g[None, :]], 0).astype(BF16)  # [65, 768]
    bhh_n2 = np.ascontiguousarray(bhh[2 * D:].reshape(2, 128).T).astype(np.float32)

    shared = {
        "wq_t": wt(Wq, scale), "wk_t": wt(Wk), "wv_t": wt(Wv), "wo_t": wt(Wo),
        "wih_aug": wih_aug, "whh_t": wt(Whh), "bhh_n2": bhh_n2,
    }
    in_maps = []
    for c in range(ncores):
        sl = slice(c * bc, (c + 1) * bc)
        in_maps.append({
            "latT": np.ascontiguousarray(latT[:, sl]),
            "commT": np.ascontiguousarray(commT[:, sl]),
            "umask": umask[sl],
            **shared,
        })
    return in_maps


def unpack_out(o, bc=BC):
    # [128, bc, 2, N] bf16 -> [bc, N, D] f32
    return o.astype(np.float32).transpose(1, 3, 2, 0).reshape(bc, N, D)


_NC_CACHE = None


def kernel(**inputs) -> np.ndarray:
    global _NC_CACHE
    from concourse.bass_utils import run_bass_kernel_spmd

    bq = np.asarray(inputs["bq"]); bk = np.asarray(inputs["bk"])
    bv = np.asarray(inputs["bv"])
    assert not np.any(bq) and not np.any(bk) and not np.any(bv), \
        "kernel assumes zero qkv biases"

    if _NC_CACHE is None:
        _NC_CACHE = build_bass()
    in_maps = prep_inputs(inputs)
    res = run_bass_kernel_spmd(_NC_CACHE, in_maps, list(range(NCORES)))
    outs = [unpack_out(res.results[c]["out_t"]) for c in range(NCORES)]
    return np.ascontiguousarray(np.concatenate(outs, 0)).astype(np.float32)
